# revision 2
# baseline (speedup 1.0000x reference)
"""GroupQueryAttention (16 heads, hd=128) on 8 trn2 cores, heads sharded 2/core.

v2: bf16 data path. x, Wq/Wk/Wv/Wo, q, k, v, probs, attnT all bf16 in
SBUF/DMA; every matmul accumulates in fp32 PSUM. Halves HBM traffic and
DVE element counts vs f32, and enables Fast Weight Load on the PE
(fp32 weights cannot FWL).

Layouts (per core c, host-prepped):
  xT    [B, 4, 16, 128, 512] bf16  x[b].T chunked: (chunk, ktile, h-part, t-col)
  wqT/wkT/wvT [16, 128, 256] bf16  W[256c:256c+256,:].T chunked by h-ktile
  woT   [2, 128, 2048] bf16        Wo[:, 256c:256c+256].T per local head
  out   [4096, 2048] f32           partial product, host sums over cores

Device per (b, h): scoresT[tk,tq] = kT.T@qT -> exp (ACT, psum->sbuf bf16) ->
PV attnT[hd,tq] = v.T-chain; rowsum via ones-col matmul over a DVE
bf16 add-tree of the 16 prob tiles; normalize attnT via PE-broadcast
reciprocal; out-proj from attnT (bf16 stationary) @ woT.
"""
import sys

for _p in ("/opt/trn_rl_repo",):
    if _p not in sys.path:
        sys.path.insert(0, _p)

import numpy as np
import ml_dtypes

import concourse.bass as bass
import concourse.tile as tile
from concourse import bacc, mybir
from concourse.bass_utils import run_bass_kernel_spmd

N_CORES = 8
B, T, H = 2, 2048, 2048
NH, HD = 16, 128
HPC = H // N_CORES          # 256 dims (2 heads) per core
HEADS_PC = NH // N_CORES    # 2
KT = H // 128               # 16 k-tiles along hidden
TCH = 4                     # t-chunks (512 cols) per batch for projections
TC = T // TCH               # 512
QC = 512                    # tq chunk in attention
NQC = T // QC               # 4
SCALE = float(HD) ** -0.5

F32 = mybir.dt.float32
F32R = mybir.dt.float32r
BF16 = mybir.dt.bfloat16
AF = mybir.ActivationFunctionType
OP = mybir.AluOpType
NPBF16 = ml_dtypes.bfloat16


def r(ap):
    return ap


_CACHE = {}


def _build(use_mask, use_bias):
    key = (use_mask, use_bias)
    if key in _CACHE:
        return _CACHE[key]

    nc = bacc.Bacc("TRN2", target_bir_lowering=False, debug=False,
                   num_devices=N_CORES)
    xT = nc.dram_tensor("xT", [B, TCH, KT, 128, TC], BF16, kind="ExternalInput").ap()
    wqT = nc.dram_tensor("wqT", [KT, 128, HPC], BF16, kind="ExternalInput").ap()
    wkT = nc.dram_tensor("wkT", [KT, 128, HPC], BF16, kind="ExternalInput").ap()
    wvT = nc.dram_tensor("wvT", [KT, 128, HPC], BF16, kind="ExternalInput").ap()
    woT = nc.dram_tensor("woT", [HEADS_PC, 128, H], BF16, kind="ExternalInput").ap()
    if use_bias:
        bqd = nc.dram_tensor("bq", [HEADS_PC, 128], F32, kind="ExternalInput").ap()
        bkd = nc.dram_tensor("bk", [HEADS_PC, 128], F32, kind="ExternalInput").ap()
        bvd = nc.dram_tensor("bv", [1, HPC], F32R, kind="ExternalInput").ap()
    if use_mask:
        # mask[b,0].T / SCALE, tk-tiled
        mkd = nc.dram_tensor("maskT", [B, KT, 128, T], F32, kind="ExternalInput").ap()
    onr = nc.dram_tensor("ones_row", [1, 128], F32R, kind="ExternalInput").ap()
    out = nc.dram_tensor("out", [B * T, H], F32, kind="ExternalOutput").ap()

    from contextlib import ExitStack
    with tile.TileContext(nc) as tc, ExitStack() as ctx:
        wpool = ctx.enter_context(tc.tile_pool(name="wts", bufs=1))
        cpool = ctx.enter_context(tc.tile_pool(name="consts", bufs=1))
        xpool = ctx.enter_context(tc.tile_pool(name="xt", bufs=2))
        qkv_pool = ctx.enter_context(tc.tile_pool(name="qkv", bufs=1))
        pr_pool = ctx.enter_context(tc.tile_pool(name="probs", bufs=4))
        acc_pool = ctx.enter_context(tc.tile_pool(name="acc", bufs=2))
        rec_pool = ctx.enter_context(tc.tile_pool(name="rec", bufs=2))
        bcs_pool = ctx.enter_context(tc.tile_pool(name="bcs", bufs=2))
        at_pool = ctx.enter_context(tc.tile_pool(name="attnT", bufs=1))
        os_pool = ctx.enter_context(tc.tile_pool(name="osb", bufs=3))
        if use_mask:
            mk_pool = ctx.enter_context(tc.tile_pool(name="mask", bufs=4))

        proj_ps = ctx.enter_context(tc.tile_pool(name="proj_ps", bufs=2, space="PSUM"))
        sc_ps = ctx.enter_context(tc.tile_pool(name="sc_ps", bufs=2, space="PSUM"))
        pv_ps = ctx.enter_context(tc.tile_pool(name="pv_ps", bufs=2, space="PSUM"))
        rb_ps = ctx.enter_context(tc.tile_pool(name="rb_ps", bufs=1, space="PSUM"))

        # ---- load weights / constants ----
        def load_w(dram):
            t = wpool.tile([128, KT * HPC], BF16, tag=dram.name)
            nc.sync.dma_start(t[:].rearrange("p (i j) -> p i j", j=HPC),
                              dram.rearrange("i p j -> p i j"))
            return t

        xt00 = xpool.tile([128, KT * TC], BF16, tag="xt", name="xt00")
        nc.sync.dma_start(xt00[:, :TC], xT[0, 0, 0])
        wq = wpool.tile([128, KT * HPC], BF16, tag="wqT", name="wq_t")
        nc.sync.dma_start(wq[:, :HPC], wqT[0])
        nc.sync.dma_start(xt00[:, TC:].rearrange("p (i j) -> p i j", j=TC),
                          xT[0, 0, 1:].rearrange("i p j -> p i j"))
        nc.sync.dma_start(wq[:, HPC:].rearrange("p (i j) -> p i j", j=HPC),
                          wqT[1:].rearrange("i p j -> p i j"))
        wk, wv = load_w(wkT), load_w(wvT)
        wo = wpool.tile([128, HEADS_PC * H], BF16, tag="wo")

        ones_col = cpool.tile([128, 1], BF16, tag="ones_col")
        nc.vector.memset(ones_col[:], 1.0)
        ones_row = cpool.tile([1, 128], F32R, tag="ones_row")
        nc.sync.dma_start(ones_row[:], onr)

        if use_bias:
            bq_t = cpool.tile([128, HEADS_PC], F32, tag="bq")
            nc.sync.dma_start(bq_t[:], bqd.rearrange("h p -> p h"))
            bk_t = cpool.tile([128, HEADS_PC], F32, tag="bk")
            nc.sync.dma_start(bk_t[:], bkd.rearrange("h p -> p h"))
            bv_row = cpool.tile([1, HPC], F32R, tag="bv_row")
            nc.sync.dma_start(bv_row[:], bvd)
            bv_ps = rb_ps.tile([128, HPC], F32, tag="rb")
            nc.tensor.matmul(bv_ps[:], r(ones_row[:]), r(bv_row[:]),
                             start=True, stop=True)
            bv_bc = cpool.tile([128, HPC], F32, tag="bv_bc")
            nc.vector.tensor_copy(bv_bc[:], bv_ps[:])

        for b in range(B):
            # ---- q/k/v projections for this batch ----
            qT = [qkv_pool.tile([128, T], BF16, tag=f"q{h}", name=f"qT{h}", bufs=2)
                  for h in range(HEADS_PC)]
            kTt = [qkv_pool.tile([128, T], BF16, tag=f"k{h}", name=f"kT{h}", bufs=2)
                   for h in range(HEADS_PC)]
            vt = qkv_pool.tile([128, KT * HPC], BF16, tag="v")  # [t-tile, d]

            for c in range(TCH):
                if b == 0 and c == 0:
                    xt = xt00
                else:
                    xt = xpool.tile([128, KT * TC], BF16, tag="xt")
                    nc.sync.dma_start(xt[:].rearrange("p (i j) -> p i j", j=TC),
                                      xT[b, c].rearrange("i p j -> p i j"))
                for h in range(HEADS_PC):
                    for w_, dst, bias_t in ((wq, qT[h], "bq"), (wk, kTt[h], "bk")):
                        ps = proj_ps.tile([128, TC], F32, tag="proj")
                        for i in range(KT):
                            nc.tensor.matmul(
                                ps[:],
                                r(w_[:, i * HPC + 128 * h: i * HPC + 128 * h + 128]),
                                r(xt[:, i * TC: (i + 1) * TC]),
                                start=(i == 0), stop=(i == KT - 1))
                        if use_bias:
                            bt = bq_t if bias_t == "bq" else bk_t
                            nc.scalar.activation(dst[:, c * TC:(c + 1) * TC], ps[:],
                                                 AF.Identity, bias=bt[:, h:h + 1])
                        else:
                            nc.vector.tensor_copy(dst[:, c * TC:(c + 1) * TC], ps[:])
                for s in range(4):  # four 128-row t-subtiles of this chunk
                    tt = 4 * c + s
                    ps = proj_ps.tile([128, HPC], F32, tag="proj")
                    for i in range(KT):
                        nc.tensor.matmul(
                            ps[:],
                            r(xt[:, i * TC + 128 * s: i * TC + 128 * s + 128]),
                            r(wv[:, i * HPC: (i + 1) * HPC]),
                            start=(i == 0), stop=(i == KT - 1))
                    if use_bias:
                        nc.vector.scalar_tensor_tensor(
                            vt[:, tt * HPC:(tt + 1) * HPC], ps[:], 1.0, bv_bc[:],
                            op0=OP.mult, op1=OP.add)
                    else:
                        nc.vector.tensor_copy(vt[:, tt * HPC:(tt + 1) * HPC], ps[:])

            if b == 0:
                nc.sync.dma_start(wo[:].rearrange("p (i j) -> p i j", j=H),
                                  woT.rearrange("i p j -> p i j"))

            # ---- attention (chunk-outer) interleaved with out-proj ----
            for ch in range(NQC):
                attnT = [at_pool.tile([128, QC], BF16, tag=f"a{h}", name=f"attnT{h}", bufs=2)
                         for h in range(HEADS_PC)]
                for h in range(HEADS_PC):
                    q_sl = r(qT[h][:, ch * QC:(ch + 1) * QC])
                    pv = pv_ps.tile([128, QC], F32, tag="pv")
                    rs = rb_ps.tile([1, QC], F32, tag="rb")
                    prs = []
                    lvl = {}  # add-tree: level -> pending tile
                    for i in range(KT):
                        sc = sc_ps.tile([128, QC], F32, tag="sc")
                        nc.tensor.matmul(sc[:], r(kTt[h][:, i * 128:(i + 1) * 128]),
                                         q_sl, start=True, stop=True)
                        if use_mask:
                            mk = mk_pool.tile([128, QC], F32, tag="mk")
                            nc.sync.dma_start(mk[:], mkd[b, i, :, ch * QC:(ch + 1) * QC])
                            nc.vector.tensor_add(sc[:], sc[:], mk[:])
                        pr = pr_pool.tile([128, QC], BF16, tag="pr")
                        nc.scalar.activation(pr[:], sc[:], AF.Exp, scale=SCALE)
                        nc.tensor.matmul(
                            pv[:],
                            r(vt[:, i * HPC + 128 * h: i * HPC + 128 * h + 128]),
                            r(pr[:]), start=(i == 0), stop=(i == KT - 1))
                        prs.append(pr)
                        # binary add-tree over the 16 prob tiles (DVE, bf16)
                        node, l = pr, 0
                        while l in lvl:
                            nxt = acc_pool.tile([128, QC], BF16, tag=f"t{l}",
                                                name=f"tree{l}")
                            nc.vector.tensor_add(nxt[:], lvl.pop(l)[:], node[:])
                            node, l = nxt, l + 1
                        lvl[l] = node
                    total = lvl[4]  # 16 tiles -> single level-4 node
                    nc.tensor.matmul(rs[:], r(ones_col[:]), r(total[:]),
                                     start=True, stop=True)
                    rec = rec_pool.tile([1, QC], F32R, tag="rec")
                    with nc.allow_low_precision(reason="f32r rowsum reciprocal"):
                        nc.vector.reciprocal(rec[:], rs[:])
                    bc = rb_ps.tile([128, QC], F32, tag="rb")
                    nc.tensor.matmul(bc[:], r(ones_row[:]), r(rec[:]),
                                     start=True, stop=True)
                    bcs = bcs_pool.tile([128, QC], F32, tag="bcs")
                    nc.vector.tensor_copy(bcs[:], bc[:])
                    nc.vector.scalar_tensor_tensor(
                        attnT[h][:], pv[:], 1.0, bcs[:],
                        op0=OP.mult, op1=OP.mult)

                # out-proj for this chunk's 4 t-tiles (partial over local dims)
                for st in range(4):
                    tt = 4 * ch + st
                    for oc in range(4):
                        ps = proj_ps.tile([128, 512], F32, tag="oproj", bufs=1)
                        for h in range(HEADS_PC):
                            nc.tensor.matmul(
                                ps[:],
                                r(attnT[h][:, st * 128:(st + 1) * 128]),
                                r(wo[:, h * H + oc * 512: h * H + (oc + 1) * 512]),
                                start=(h == 0), stop=(h == HEADS_PC - 1))
                        ob = os_pool.tile([128, 512], F32, tag="ob")
                        if (st + oc) % 2 == 0:
                            nc.vector.tensor_copy(ob[:], ps[:])
                        else:
                            nc.scalar.copy(ob[:], ps[:])
                        nc.gpsimd.dma_start(
                            out[b * T + tt * 128: b * T + (tt + 1) * 128,
                                oc * 512:(oc + 1) * 512], ob[:])


    nc.compile()
    _CACHE[key] = nc
    return nc


def prepare(inputs):
    hs = np.ascontiguousarray(np.asarray(inputs["hidden_states"], dtype=np.float32))
    mask = np.asarray(inputs["attention_mask"], dtype=np.float32)
    Wq = np.asarray(inputs["Wq"], dtype=np.float32)
    Wk = np.asarray(inputs["Wk"], dtype=np.float32)
    Wv = np.asarray(inputs["Wv"], dtype=np.float32)
    Wo = np.asarray(inputs["Wo"], dtype=np.float32)
    bq = np.asarray(inputs["bq"], dtype=np.float32)
    bk = np.asarray(inputs["bk"], dtype=np.float32)
    bv = np.asarray(inputs["bv"], dtype=np.float32)

    use_mask = bool(np.any(mask))
    use_bias = bool(np.any(bq) or np.any(bk) or np.any(bv))
    nc = _build(use_mask, use_bias)

    # x[b].T -> [h,t] -> (16,128, 4,512) -> [4,16,128,512]
    xTh = hs.transpose(0, 2, 1).reshape(B, KT, 128, TCH, TC)
    xTh = np.ascontiguousarray(xTh.transpose(0, 3, 1, 2, 4)).astype(NPBF16)

    in_maps = []
    for c in range(N_CORES):
        sl = slice(c * HPC, (c + 1) * HPC)
        m = {
            "ones_row": np.ones((1, 128), np.float32),
            "xT": xTh,
            "wqT": np.ascontiguousarray(Wq[sl].T).reshape(KT, 128, HPC).astype(NPBF16),
            "wkT": np.ascontiguousarray(Wk[sl].T).reshape(KT, 128, HPC).astype(NPBF16),
            "wvT": np.ascontiguousarray(Wv[sl].T).reshape(KT, 128, HPC).astype(NPBF16),
            "woT": np.ascontiguousarray(Wo[:, sl].T).reshape(HEADS_PC, 128, H).astype(NPBF16),
        }
        if use_bias:
            m["bq"] = np.ascontiguousarray(bq[sl]).reshape(HEADS_PC, 128)
            m["bk"] = np.ascontiguousarray(bk[sl]).reshape(HEADS_PC, 128)
            m["bv"] = np.ascontiguousarray(bv[sl]).reshape(1, HPC)
        if use_mask:
            mt = mask[:, 0].transpose(0, 2, 1) / SCALE  # [B, tk, tq]
            m["maskT"] = np.ascontiguousarray(mt).reshape(B, KT, 128, T)
        in_maps.append(m)
    return nc, in_maps


def postprocess(results, inputs):
    bo = np.asarray(inputs["bo"], dtype=np.float32)
    acc = results[0]["out"].astype(np.float32)
    for c in range(1, N_CORES):
        acc = acc + results[c]["out"]
    return (acc + bo).reshape(B, T, H)


def kernel(**inputs):
    nc, in_maps = prepare(inputs)
    res = run_bass_kernel_spmd(nc, in_maps, list(range(N_CORES)))
    return postprocess(res.results, inputs)


# revision 13
# speedup vs baseline: 1.4392x; 1.4392x over previous
"""GroupQueryAttention (16 heads, hd=128) on 8 trn2 cores, heads sharded 2/core.

v2: bf16 data path. x, Wq/Wk/Wv/Wo, q, k, v, probs, attnT all bf16 in
SBUF/DMA; every matmul accumulates in fp32 PSUM. Halves HBM traffic and
DVE element counts vs f32, and enables Fast Weight Load on the PE
(fp32 weights cannot FWL).

Layouts (per core c, host-prepped):
  xT    [B, 4, 16, 128, 512] bf16  x[b].T chunked: (chunk, ktile, h-part, t-col)
  wqT/wkT/wvT [16, 128, 256] bf16  W[256c:256c+256,:].T chunked by h-ktile
  woT   [2, 128, 2048] bf16        Wo[:, 256c:256c+256].T per local head
  out   [4096, 2048] f32           partial product, host sums over cores

Device per (b, h): scoresT[tk,tq] = kT.T@qT -> exp (ACT, psum->sbuf bf16) ->
PV attnT[hd,tq] = v.T-chain; rowsum via ones-col matmul over a DVE
bf16 add-tree of the 16 prob tiles; normalize attnT via PE-broadcast
reciprocal; out-proj from attnT (bf16 stationary) @ woT.
"""
import sys

for _p in ("/opt/trn_rl_repo",):
    if _p not in sys.path:
        sys.path.insert(0, _p)

import numpy as np
import ml_dtypes

import concourse.bass as bass
import concourse.tile as tile
from concourse import bacc, mybir
from concourse.bass_utils import run_bass_kernel_spmd

N_CORES = 8
B, T, H = 2, 2048, 2048
NH, HD = 16, 128
HPC = H // N_CORES          # 256 dims (2 heads) per core
HEADS_PC = NH // N_CORES    # 2
KT = H // 128               # 16 k-tiles along hidden
TCH = 4                     # t-chunks (512 cols) per batch for projections
TC = T // TCH               # 512
QC = 512                    # tq chunk in attention
NQC = T // QC               # 4
SCALE = float(HD) ** -0.5
XT_SZ = B * TCH * KT * 128 * TC      # hidden_states.T, chunked
W_SZ = KT * 128 * HPC                # one projection weight slice
BLOB_SZ = XT_SZ + 4 * W_SZ

F32 = mybir.dt.float32
F32R = mybir.dt.float32r
BF16 = mybir.dt.bfloat16
AF = mybir.ActivationFunctionType
OP = mybir.AluOpType
NPBF16 = ml_dtypes.bfloat16


def r(ap):
    return ap


_CACHE = {}


def _build(use_mask, use_bias):
    key = (use_mask, use_bias)
    if key in _CACHE:
        return _CACHE[key]

    nc = bacc.Bacc("TRN2", target_bir_lowering=False, debug=False,
                   num_devices=N_CORES)
    # all inputs packed into one blob: per-call dispatch cost through the
    # axon tunnel scales with operand count (~37us/operand), so one buffer
    # beats six
    blob = nc.dram_tensor("blob", [BLOB_SZ], BF16, kind="ExternalInput").ap()
    xT = blob[0:XT_SZ].rearrange("(b c i p j) -> b c i p j",
                                 b=B, c=TCH, i=KT, p=128, j=TC)
    _o = XT_SZ
    wqT = blob[_o:_o + W_SZ].rearrange("(i p j) -> i p j", i=KT, p=128, j=HPC)
    _o += W_SZ
    wkT = blob[_o:_o + W_SZ].rearrange("(i p j) -> i p j", i=KT, p=128, j=HPC)
    _o += W_SZ
    wvT = blob[_o:_o + W_SZ].rearrange("(i p j) -> i p j", i=KT, p=128, j=HPC)
    _o += W_SZ
    woT = blob[_o:_o + W_SZ].rearrange("(h p j) -> h p j", h=HEADS_PC, p=128, j=H)
    if use_bias:
        bqd = nc.dram_tensor("bq", [HEADS_PC, 128], F32, kind="ExternalInput").ap()
        bkd = nc.dram_tensor("bk", [HEADS_PC, 128], F32, kind="ExternalInput").ap()
        bvd = nc.dram_tensor("bv", [1, HPC], F32R, kind="ExternalInput").ap()
    if use_mask:
        # mask[b,0].T / SCALE, tk-tiled
        mkd = nc.dram_tensor("maskT", [B, KT, 128, T], F32, kind="ExternalInput").ap()
    out = nc.dram_tensor("out", [B * T, H], BF16, kind="ExternalOutput").ap()

    from contextlib import ExitStack
    with tile.TileContext(nc) as tc, ExitStack() as ctx:
        wpool = ctx.enter_context(tc.tile_pool(name="wts", bufs=1))
        cpool = ctx.enter_context(tc.tile_pool(name="consts", bufs=1))
        xpool = ctx.enter_context(tc.tile_pool(name="xt", bufs=2))
        qkv_pool = ctx.enter_context(tc.tile_pool(name="qkv", bufs=1))
        pr_pool = ctx.enter_context(tc.tile_pool(name="probs", bufs=4))
        acc_pool = ctx.enter_context(tc.tile_pool(name="acc", bufs=2))
        rec_pool = ctx.enter_context(tc.tile_pool(name="rec", bufs=2))
        bcs_pool = ctx.enter_context(tc.tile_pool(name="bcs", bufs=2))
        at_pool = ctx.enter_context(tc.tile_pool(name="attnT", bufs=1))
        os_pool = ctx.enter_context(tc.tile_pool(name="osb", bufs=3))
        if use_mask:
            mk_pool = ctx.enter_context(tc.tile_pool(name="mask", bufs=4))

        proj_ps = ctx.enter_context(tc.tile_pool(name="proj_ps", bufs=2, space="PSUM"))
        sc_ps = ctx.enter_context(tc.tile_pool(name="sc_ps", bufs=2, space="PSUM"))
        pv_ps = ctx.enter_context(tc.tile_pool(name="pv_ps", bufs=2, space="PSUM"))
        rb_ps = ctx.enter_context(tc.tile_pool(name="rb_ps", bufs=1, space="PSUM"))

        # ---- load weights / constants ----
        def load_w(dram, tag):
            t = wpool.tile([128, KT * HPC], BF16, tag=tag)
            nc.sync.dma_start(t[:].rearrange("p (i j) -> p i j", j=HPC),
                              dram.rearrange("i p j -> p i j"))
            return t

        xt00 = xpool.tile([128, KT * TC], BF16, tag="xt", name="xt00")
        nc.sync.dma_start(xt00[:, :TC], xT[0, 0, 0])
        wq = wpool.tile([128, KT * HPC], BF16, tag="wqT", name="wq_t")
        nc.sync.dma_start(wq[:, :HPC], wqT[0])
        nc.sync.dma_start(xt00[:, TC:].rearrange("p (i j) -> p i j", j=TC),
                          xT[0, 0, 1:].rearrange("i p j -> p i j"))
        nc.sync.dma_start(wq[:, HPC:].rearrange("p (i j) -> p i j", j=HPC),
                          wqT[1:].rearrange("i p j -> p i j"))
        wk, wv = load_w(wkT, "wk"), load_w(wvT, "wv")
        wo = wpool.tile([128, HEADS_PC * H], BF16, tag="wo")

        ones_col = cpool.tile([128, 1], BF16, tag="ones_col")
        nc.vector.memset(ones_col[:], 1.0)
        # memset can't write f32r directly (ISA reject): stage via f32
        ones_row_f = cpool.tile([1, 128], F32, tag="ones_row_f")
        nc.vector.memset(ones_row_f[:], 1.0)
        ones_row = cpool.tile([1, 128], F32R, tag="ones_row")
        nc.vector.tensor_copy(ones_row[:], ones_row_f[:])

        if use_bias:
            bq_t = cpool.tile([128, HEADS_PC], F32, tag="bq")
            nc.sync.dma_start(bq_t[:], bqd.rearrange("h p -> p h"))
            bk_t = cpool.tile([128, HEADS_PC], F32, tag="bk")
            nc.sync.dma_start(bk_t[:], bkd.rearrange("h p -> p h"))
            bv_row = cpool.tile([1, HPC], F32R, tag="bv_row")
            nc.sync.dma_start(bv_row[:], bvd)
            bv_ps = rb_ps.tile([128, HPC], F32, tag="rb")
            nc.tensor.matmul(bv_ps[:], r(ones_row[:]), r(bv_row[:]),
                             start=True, stop=True)
            bv_bc = cpool.tile([128, HPC], F32, tag="bv_bc")
            nc.vector.tensor_copy(bv_bc[:], bv_ps[:])

        for b in range(B):
            # ---- q/k/v projections for this batch ----
            qT = [qkv_pool.tile([128, T], BF16, tag=f"q{h}", name=f"qT{h}", bufs=2)
                  for h in range(HEADS_PC)]
            kTt = [qkv_pool.tile([128, T], BF16, tag=f"k{h}", name=f"kT{h}", bufs=2)
                   for h in range(HEADS_PC)]
            vt = qkv_pool.tile([128, KT * HPC], BF16, tag="v")  # [t-tile, d]

            for c in range(TCH):
                if b == 0 and c == 0:
                    xt = xt00
                else:
                    xt = xpool.tile([128, KT * TC], BF16, tag="xt")
                    nc.sync.dma_start(xt[:].rearrange("p (i j) -> p i j", j=TC),
                                      xT[b, c].rearrange("i p j -> p i j"))
                for h in range(HEADS_PC):
                    for w_, dst, bias_t in ((wq, qT[h], "bq"), (wk, kTt[h], "bk")):
                        ps = proj_ps.tile([128, TC], F32, tag="proj")
                        for i in range(KT):
                            nc.tensor.matmul(
                                ps[:],
                                r(w_[:, i * HPC + 128 * h: i * HPC + 128 * h + 128]),
                                r(xt[:, i * TC: (i + 1) * TC]),
                                start=(i == 0), stop=(i == KT - 1))
                        if use_bias:
                            bt = bq_t if bias_t == "bq" else bk_t
                            nc.scalar.activation(dst[:, c * TC:(c + 1) * TC], ps[:],
                                                 AF.Identity, bias=bt[:, h:h + 1])
                        else:
                            nc.vector.tensor_copy(dst[:, c * TC:(c + 1) * TC], ps[:])
                for s in range(4):  # four 128-row t-subtiles of this chunk
                    tt = 4 * c + s
                    ps = proj_ps.tile([128, HPC], F32, tag="proj")
                    for i in range(KT):
                        nc.tensor.matmul(
                            ps[:],
                            r(xt[:, i * TC + 128 * s: i * TC + 128 * s + 128]),
                            r(wv[:, i * HPC: (i + 1) * HPC]),
                            start=(i == 0), stop=(i == KT - 1))
                    if use_bias:
                        nc.vector.scalar_tensor_tensor(
                            vt[:, tt * HPC:(tt + 1) * HPC], ps[:], 1.0, bv_bc[:],
                            op0=OP.mult, op1=OP.add)
                    else:
                        nc.vector.tensor_copy(vt[:, tt * HPC:(tt + 1) * HPC], ps[:])

            if b == 0:
                nc.sync.dma_start(wo[:].rearrange("p (i j) -> p i j", j=H),
                                  woT.rearrange("i p j -> p i j"))

            # ---- attention (chunk-outer) interleaved with out-proj ----
            for ch in range(NQC):
                attnT = [at_pool.tile([128, QC], BF16, tag=f"a{h}", name=f"attnT{h}", bufs=2)
                         for h in range(HEADS_PC)]
                for h in range(HEADS_PC):
                    q_sl = r(qT[h][:, ch * QC:(ch + 1) * QC])
                    pv = pv_ps.tile([128, QC], F32, tag="pv")
                    rs = rb_ps.tile([1, QC], F32, tag="rb")
                    prs = []
                    lvl = {}  # add-tree: level -> pending tile
                    for i in range(KT):
                        sc = sc_ps.tile([128, QC], F32, tag="sc")
                        nc.tensor.matmul(sc[:], r(kTt[h][:, i * 128:(i + 1) * 128]),
                                         q_sl, start=True, stop=True)
                        if use_mask:
                            mk = mk_pool.tile([128, QC], F32, tag="mk")
                            nc.sync.dma_start(mk[:], mkd[b, i, :, ch * QC:(ch + 1) * QC])
                            nc.vector.tensor_add(sc[:], sc[:], mk[:])
                        pr = pr_pool.tile([128, QC], BF16, tag="pr")
                        nc.scalar.activation(pr[:], sc[:], AF.Exp, scale=SCALE)
                        nc.tensor.matmul(
                            pv[:],
                            r(vt[:, i * HPC + 128 * h: i * HPC + 128 * h + 128]),
                            r(pr[:]), start=(i == 0), stop=(i == KT - 1))
                        prs.append(pr)
                        # binary add-tree over the 16 prob tiles (DVE, bf16)
                        node, l = pr, 0
                        while l in lvl:
                            nxt = acc_pool.tile([128, QC], BF16, tag=f"t{l}",
                                                name=f"tree{l}")
                            nc.vector.tensor_add(nxt[:], lvl.pop(l)[:], node[:])
                            node, l = nxt, l + 1
                        lvl[l] = node
                    total = lvl[4]  # 16 tiles -> single level-4 node
                    nc.tensor.matmul(rs[:], r(ones_col[:]), r(total[:]),
                                     start=True, stop=True)
                    rec = rec_pool.tile([1, QC], F32R, tag="rec")
                    with nc.allow_low_precision(reason="f32r rowsum reciprocal"):
                        nc.vector.reciprocal(rec[:], rs[:])
                    bc = rb_ps.tile([128, QC], F32, tag="rb")
                    nc.tensor.matmul(bc[:], r(ones_row[:]), r(rec[:]),
                                     start=True, stop=True)
                    bcs = bcs_pool.tile([128, QC], F32, tag="bcs")
                    nc.vector.tensor_copy(bcs[:], bc[:])
                    nc.vector.scalar_tensor_tensor(
                        attnT[h][:], pv[:], 1.0, bcs[:],
                        op0=OP.mult, op1=OP.mult)

                # out-proj for this chunk's 4 t-tiles (partial over local dims)
                for st in range(4):
                    tt = 4 * ch + st
                    for oc in range(4):
                        ps = proj_ps.tile([128, 512], F32, tag="oproj", bufs=1)
                        for h in range(HEADS_PC):
                            nc.tensor.matmul(
                                ps[:],
                                r(attnT[h][:, st * 128:(st + 1) * 128]),
                                r(wo[:, h * H + oc * 512: h * H + (oc + 1) * 512]),
                                start=(h == 0), stop=(h == HEADS_PC - 1))
                        ob = os_pool.tile([128, 512], BF16, tag="ob")
                        if (st + oc) % 2 == 0:
                            nc.vector.tensor_copy(ob[:], ps[:])
                        else:
                            nc.scalar.copy(ob[:], ps[:])
                        nc.gpsimd.dma_start(
                            out[b * T + tt * 128: b * T + (tt + 1) * 128,
                                oc * 512:(oc + 1) * 512], ob[:])


    nc.compile()
    _CACHE[key] = nc
    return nc


def prepare(inputs):
    hs = np.ascontiguousarray(np.asarray(inputs["hidden_states"], dtype=np.float32))
    mask = np.asarray(inputs["attention_mask"], dtype=np.float32)
    Wq = np.asarray(inputs["Wq"], dtype=np.float32)
    Wk = np.asarray(inputs["Wk"], dtype=np.float32)
    Wv = np.asarray(inputs["Wv"], dtype=np.float32)
    Wo = np.asarray(inputs["Wo"], dtype=np.float32)
    bq = np.asarray(inputs["bq"], dtype=np.float32)
    bk = np.asarray(inputs["bk"], dtype=np.float32)
    bv = np.asarray(inputs["bv"], dtype=np.float32)

    use_mask = bool(np.any(mask))
    use_bias = bool(np.any(bq) or np.any(bk) or np.any(bv))
    nc = _build(use_mask, use_bias)

    # x[b].T -> [h,t] -> (16,128, 4,512) -> [4,16,128,512]
    xTh = hs.transpose(0, 2, 1).reshape(B, KT, 128, TCH, TC)
    xTh = np.ascontiguousarray(xTh.transpose(0, 3, 1, 2, 4)).astype(NPBF16)
    xflat = xTh.reshape(-1)

    in_maps = []
    for c in range(N_CORES):
        sl = slice(c * HPC, (c + 1) * HPC)
        blob = np.concatenate([
            xflat,
            np.ascontiguousarray(Wq[sl].T).astype(NPBF16).reshape(-1),
            np.ascontiguousarray(Wk[sl].T).astype(NPBF16).reshape(-1),
            np.ascontiguousarray(Wv[sl].T).astype(NPBF16).reshape(-1),
            np.ascontiguousarray(Wo[:, sl].T).astype(NPBF16).reshape(-1),
        ])
        assert blob.shape == (BLOB_SZ,)
        m = {"blob": blob}
        if use_bias:
            m["bq"] = np.ascontiguousarray(bq[sl]).reshape(HEADS_PC, 128)
            m["bk"] = np.ascontiguousarray(bk[sl]).reshape(HEADS_PC, 128)
            m["bv"] = np.ascontiguousarray(bv[sl]).reshape(1, HPC)
        if use_mask:
            mt = mask[:, 0].transpose(0, 2, 1) / SCALE  # [B, tk, tq]
            m["maskT"] = np.ascontiguousarray(mt).reshape(B, KT, 128, T)
        in_maps.append(m)
    return nc, in_maps


def postprocess(results, inputs):
    bo = np.asarray(inputs["bo"], dtype=np.float32)
    acc = results[0]["out"].astype(np.float32)
    for c in range(1, N_CORES):
        acc = acc + results[c]["out"].astype(np.float32)
    return (acc + bo).reshape(B, T, H)


def kernel(**inputs):
    import time as _time

    nc, in_maps = prepare(inputs)
    last_err = None
    for attempt in range(3):
        try:
            res = run_bass_kernel_spmd(nc, in_maps, list(range(N_CORES)))
            result = postprocess(res.results, inputs)
            if np.isfinite(result).all():
                return result
            # transient device fault can yield garbage without raising
            last_err = ValueError("non-finite kernel output")
        except Exception as e:
            last_err = e
        _time.sleep(2.0)
    raise last_err


# revision 14
# speedup vs baseline: 1.4699x; 1.0213x over previous
"""GroupQueryAttention (16 heads, hd=128) on 8 trn2 cores, heads sharded 2/core.

v2: bf16 data path. x, Wq/Wk/Wv/Wo, q, k, v, probs, attnT all bf16 in
SBUF/DMA; every matmul accumulates in fp32 PSUM. Halves HBM traffic and
DVE element counts vs f32, and enables Fast Weight Load on the PE
(fp32 weights cannot FWL).

Layouts (per core c, host-prepped):
  xT    [B, 4, 16, 128, 512] bf16  x[b].T chunked: (chunk, ktile, h-part, t-col)
  wqT/wkT/wvT [16, 128, 256] bf16  W[256c:256c+256,:].T chunked by h-ktile
  woT   [2, 128, 2048] bf16        Wo[:, 256c:256c+256].T per local head
  out   [4096, 2048] f32           partial product, host sums over cores

Device per (b, h): scoresT[tk,tq] = kT.T@qT -> exp (ACT, psum->sbuf bf16) ->
PV attnT[hd,tq] = v.T-chain; rowsum via ones-col matmul over a DVE
bf16 add-tree of the 16 prob tiles; normalize attnT via PE-broadcast
reciprocal; out-proj from attnT (bf16 stationary) @ woT.
"""
import sys

for _p in ("/opt/trn_rl_repo",):
    if _p not in sys.path:
        sys.path.insert(0, _p)

import numpy as np
import ml_dtypes

import concourse.bass as bass
import concourse.tile as tile
from concourse import bacc, mybir
from concourse.bass_utils import run_bass_kernel_spmd

N_CORES = 8
B, T, H = 2, 2048, 2048
NH, HD = 16, 128
HPC = H // N_CORES          # 256 dims (2 heads) per core
HEADS_PC = NH // N_CORES    # 2
KT = H // 128               # 16 k-tiles along hidden
TCH = 4                     # t-chunks (512 cols) per batch for projections
TC = T // TCH               # 512
QC = 512                    # tq chunk in attention
NQC = T // QC               # 4
SCALE = float(HD) ** -0.5
XT_SZ = B * TCH * KT * 128 * TC      # hidden_states.T, chunked
W_SZ = KT * 128 * HPC                # one projection weight slice
BLOB_SZ = XT_SZ + 4 * W_SZ

F32 = mybir.dt.float32
F32R = mybir.dt.float32r
BF16 = mybir.dt.bfloat16
AF = mybir.ActivationFunctionType
OP = mybir.AluOpType
NPBF16 = ml_dtypes.bfloat16


def r(ap):
    return ap


_CACHE = {}


def _build(use_mask, use_bias):
    key = (use_mask, use_bias)
    if key in _CACHE:
        return _CACHE[key]

    nc = bacc.Bacc("TRN2", target_bir_lowering=False, debug=False,
                   num_devices=N_CORES)
    # all inputs packed into one blob: per-call dispatch cost through the
    # axon tunnel scales with operand count (~37us/operand), so one buffer
    # beats six
    blob = nc.dram_tensor("blob", [BLOB_SZ], BF16, kind="ExternalInput").ap()
    xT = blob[0:XT_SZ].rearrange("(b c i p j) -> b c i p j",
                                 b=B, c=TCH, i=KT, p=128, j=TC)
    _o = XT_SZ
    wqT = blob[_o:_o + W_SZ].rearrange("(i p j) -> i p j", i=KT, p=128, j=HPC)
    _o += W_SZ
    wkT = blob[_o:_o + W_SZ].rearrange("(i p j) -> i p j", i=KT, p=128, j=HPC)
    _o += W_SZ
    wvT = blob[_o:_o + W_SZ].rearrange("(i p j) -> i p j", i=KT, p=128, j=HPC)
    _o += W_SZ
    woT = blob[_o:_o + W_SZ].rearrange("(h p j) -> h p j", h=HEADS_PC, p=128, j=H)
    if use_bias:
        bqd = nc.dram_tensor("bq", [HEADS_PC, 128], F32, kind="ExternalInput").ap()
        bkd = nc.dram_tensor("bk", [HEADS_PC, 128], F32, kind="ExternalInput").ap()
        bvd = nc.dram_tensor("bv", [1, HPC], F32R, kind="ExternalInput").ap()
    if use_mask:
        # mask[b,0].T / SCALE, tk-tiled
        mkd = nc.dram_tensor("maskT", [B, KT, 128, T], F32, kind="ExternalInput").ap()
    out = nc.dram_tensor("out", [B * T, H], BF16, kind="ExternalOutput").ap()

    from contextlib import ExitStack
    with tile.TileContext(nc) as tc, ExitStack() as ctx:
        wpool = ctx.enter_context(tc.tile_pool(name="wts", bufs=1))
        cpool = ctx.enter_context(tc.tile_pool(name="consts", bufs=1))
        xpool = ctx.enter_context(tc.tile_pool(name="xt", bufs=2))
        qkv_pool = ctx.enter_context(tc.tile_pool(name="qkv", bufs=1))
        pr_pool = ctx.enter_context(tc.tile_pool(name="probs", bufs=4))
        acc_pool = ctx.enter_context(tc.tile_pool(name="acc", bufs=2))
        rec_pool = ctx.enter_context(tc.tile_pool(name="rec", bufs=2))
        bcs_pool = ctx.enter_context(tc.tile_pool(name="bcs", bufs=2))
        at_pool = ctx.enter_context(tc.tile_pool(name="attnT", bufs=1))
        os_pool = ctx.enter_context(tc.tile_pool(name="osb", bufs=3))
        if use_mask:
            mk_pool = ctx.enter_context(tc.tile_pool(name="mask", bufs=4))

        proj_ps = ctx.enter_context(tc.tile_pool(name="proj_ps", bufs=2, space="PSUM"))
        sc_ps = ctx.enter_context(tc.tile_pool(name="sc_ps", bufs=2, space="PSUM"))
        pv_ps = ctx.enter_context(tc.tile_pool(name="pv_ps", bufs=2, space="PSUM"))
        rb_ps = ctx.enter_context(tc.tile_pool(name="rb_ps", bufs=1, space="PSUM"))

        # ---- load weights / constants ----
        def load_w(dram, tag):
            t = wpool.tile([128, KT * HPC], BF16, tag=tag)
            nc.sync.dma_start(t[:].rearrange("p (i j) -> p i j", j=HPC),
                              dram.rearrange("i p j -> p i j"))
            return t

        xt00 = xpool.tile([128, KT * TC], BF16, tag="xt", name="xt00")
        nc.sync.dma_start(xt00[:, :TC], xT[0, 0, 0])
        wq = wpool.tile([128, KT * HPC], BF16, tag="wqT", name="wq_t")
        nc.sync.dma_start(wq[:, :HPC], wqT[0])
        nc.sync.dma_start(xt00[:, TC:].rearrange("p (i j) -> p i j", j=TC),
                          xT[0, 0, 1:].rearrange("i p j -> p i j"))
        nc.sync.dma_start(wq[:, HPC:].rearrange("p (i j) -> p i j", j=HPC),
                          wqT[1:].rearrange("i p j -> p i j"))
        wk, wv = load_w(wkT, "wk"), load_w(wvT, "wv")
        wo = wpool.tile([128, HEADS_PC * H], BF16, tag="wo")

        ones_col = cpool.tile([128, 1], BF16, tag="ones_col")
        nc.vector.memset(ones_col[:], 1.0)
        # memset can't write f32r directly (ISA reject): stage via f32
        ones_row_f = cpool.tile([1, 128], F32, tag="ones_row_f")
        nc.vector.memset(ones_row_f[:], 1.0)
        ones_row = cpool.tile([1, 128], F32R, tag="ones_row")
        nc.vector.tensor_copy(ones_row[:], ones_row_f[:])

        if use_bias:
            bq_t = cpool.tile([128, HEADS_PC], F32, tag="bq")
            nc.sync.dma_start(bq_t[:], bqd.rearrange("h p -> p h"))
            bk_t = cpool.tile([128, HEADS_PC], F32, tag="bk")
            nc.sync.dma_start(bk_t[:], bkd.rearrange("h p -> p h"))
            bv_row = cpool.tile([1, HPC], F32R, tag="bv_row")
            nc.sync.dma_start(bv_row[:], bvd)
            bv_ps = rb_ps.tile([128, HPC], F32, tag="rb")
            nc.tensor.matmul(bv_ps[:], r(ones_row[:]), r(bv_row[:]),
                             start=True, stop=True)
            bv_bc = cpool.tile([128, HPC], F32, tag="bv_bc")
            nc.vector.tensor_copy(bv_bc[:], bv_ps[:])

        for b in range(B):
            # ---- q/k/v projections for this batch ----
            qT = [qkv_pool.tile([128, T], BF16, tag=f"q{h}", name=f"qT{h}", bufs=2)
                  for h in range(HEADS_PC)]
            kTt = [qkv_pool.tile([128, T], BF16, tag=f"k{h}", name=f"kT{h}", bufs=2)
                   for h in range(HEADS_PC)]
            vt = qkv_pool.tile([128, KT * HPC], BF16, tag="v")  # [t-tile, d]

            for c in range(TCH):
                if b == 0 and c == 0:
                    xt = xt00
                else:
                    xt = xpool.tile([128, KT * TC], BF16, tag="xt")
                    nc.sync.dma_start(xt[:].rearrange("p (i j) -> p i j", j=TC),
                                      xT[b, c].rearrange("i p j -> p i j"))
                for h in range(HEADS_PC):
                    for w_, dst, bias_t in ((wq, qT[h], "bq"), (wk, kTt[h], "bk")):
                        ps = proj_ps.tile([128, TC], F32, tag="proj")
                        for i in range(KT):
                            nc.tensor.matmul(
                                ps[:],
                                r(w_[:, i * HPC + 128 * h: i * HPC + 128 * h + 128]),
                                r(xt[:, i * TC: (i + 1) * TC]),
                                start=(i == 0), stop=(i == KT - 1))
                        if use_bias:
                            bt = bq_t if bias_t == "bq" else bk_t
                            nc.scalar.activation(dst[:, c * TC:(c + 1) * TC], ps[:],
                                                 AF.Identity, bias=bt[:, h:h + 1])
                        else:
                            nc.vector.tensor_copy(dst[:, c * TC:(c + 1) * TC], ps[:])
                for s in range(4):  # four 128-row t-subtiles of this chunk
                    tt = 4 * c + s
                    ps = proj_ps.tile([128, HPC], F32, tag="proj")
                    for i in range(KT):
                        nc.tensor.matmul(
                            ps[:],
                            r(xt[:, i * TC + 128 * s: i * TC + 128 * s + 128]),
                            r(wv[:, i * HPC: (i + 1) * HPC]),
                            start=(i == 0), stop=(i == KT - 1))
                    if use_bias:
                        nc.vector.scalar_tensor_tensor(
                            vt[:, tt * HPC:(tt + 1) * HPC], ps[:], 1.0, bv_bc[:],
                            op0=OP.mult, op1=OP.add)
                    else:
                        nc.vector.tensor_copy(vt[:, tt * HPC:(tt + 1) * HPC], ps[:])

            if b == 0:
                nc.sync.dma_start(wo[:].rearrange("p (i j) -> p i j", j=H),
                                  woT.rearrange("i p j -> p i j"))

            # ---- attention (chunk-outer) interleaved with out-proj ----
            for ch in range(NQC):
                attnT = [at_pool.tile([128, QC], BF16, tag=f"a{h}", name=f"attnT{h}", bufs=2)
                         for h in range(HEADS_PC)]
                for h in range(HEADS_PC):
                    q_sl = r(qT[h][:, ch * QC:(ch + 1) * QC])
                    pv = pv_ps.tile([128, QC], F32, tag="pv")
                    rs = rb_ps.tile([1, QC], F32, tag="rb")
                    prs = []
                    lvl = {}  # add-tree: level -> pending tile
                    for i in range(KT):
                        sc = sc_ps.tile([128, QC], F32, tag="sc")
                        nc.tensor.matmul(sc[:], r(kTt[h][:, i * 128:(i + 1) * 128]),
                                         q_sl, start=True, stop=True)
                        if use_mask:
                            mk = mk_pool.tile([128, QC], F32, tag="mk")
                            nc.sync.dma_start(mk[:], mkd[b, i, :, ch * QC:(ch + 1) * QC])
                            nc.vector.tensor_add(sc[:], sc[:], mk[:])
                        pr = pr_pool.tile([128, QC], BF16, tag="pr")
                        nc.scalar.activation(pr[:], sc[:], AF.Exp, scale=SCALE)
                        nc.tensor.matmul(
                            pv[:],
                            r(vt[:, i * HPC + 128 * h: i * HPC + 128 * h + 128]),
                            r(pr[:]), start=(i == 0), stop=(i == KT - 1))
                        prs.append(pr)
                        # binary add-tree over the 16 prob tiles (DVE, bf16)
                        node, l = pr, 0
                        while l in lvl:
                            nxt = acc_pool.tile([128, QC], BF16, tag=f"t{l}",
                                                name=f"tree{l}")
                            nc.vector.tensor_add(nxt[:], lvl.pop(l)[:], node[:])
                            node, l = nxt, l + 1
                        lvl[l] = node
                    total = lvl[4]  # 16 tiles -> single level-4 node
                    nc.tensor.matmul(rs[:], r(ones_col[:]), r(total[:]),
                                     start=True, stop=True)
                    rec = rec_pool.tile([1, QC], F32R, tag="rec")
                    with nc.allow_low_precision(reason="f32r rowsum reciprocal"):
                        nc.vector.reciprocal(rec[:], rs[:])
                    bc = rb_ps.tile([128, QC], F32, tag="rb")
                    nc.tensor.matmul(bc[:], r(ones_row[:]), r(rec[:]),
                                     start=True, stop=True)
                    bcs = bcs_pool.tile([128, QC], F32, tag="bcs")
                    nc.vector.tensor_copy(bcs[:], bc[:])
                    nc.vector.scalar_tensor_tensor(
                        attnT[h][:], pv[:], 1.0, bcs[:],
                        op0=OP.mult, op1=OP.mult)

                # out-proj for this chunk's 4 t-tiles (partial over local dims)
                for st in range(4):
                    tt = 4 * ch + st
                    for oc in range(4):
                        ps = proj_ps.tile([128, 512], F32, tag="oproj", bufs=1)
                        for h in range(HEADS_PC):
                            nc.tensor.matmul(
                                ps[:],
                                r(attnT[h][:, st * 128:(st + 1) * 128]),
                                r(wo[:, h * H + oc * 512: h * H + (oc + 1) * 512]),
                                start=(h == 0), stop=(h == HEADS_PC - 1))
                        ob = os_pool.tile([128, 512], BF16, tag="ob")
                        if (st + oc) % 2 == 0:
                            nc.vector.tensor_copy(ob[:], ps[:])
                        else:
                            nc.scalar.copy(ob[:], ps[:])
                        nc.gpsimd.dma_start(
                            out[b * T + tt * 128: b * T + (tt + 1) * 128,
                                oc * 512:(oc + 1) * 512], ob[:])


    nc.compile()
    _CACHE[key] = nc
    return nc


def prepare(inputs):
    hs = np.ascontiguousarray(np.asarray(inputs["hidden_states"], dtype=np.float32))
    mask = np.asarray(inputs["attention_mask"], dtype=np.float32)
    Wq = np.asarray(inputs["Wq"], dtype=np.float32)
    Wk = np.asarray(inputs["Wk"], dtype=np.float32)
    Wv = np.asarray(inputs["Wv"], dtype=np.float32)
    Wo = np.asarray(inputs["Wo"], dtype=np.float32)
    bq = np.asarray(inputs["bq"], dtype=np.float32)
    bk = np.asarray(inputs["bk"], dtype=np.float32)
    bv = np.asarray(inputs["bv"], dtype=np.float32)

    use_mask = bool(np.any(mask))
    use_bias = bool(np.any(bq) or np.any(bk) or np.any(bv))
    nc = _build(use_mask, use_bias)

    # x[b].T -> [h,t] -> (16,128, 4,512) -> [4,16,128,512]
    xTh = hs.transpose(0, 2, 1).reshape(B, KT, 128, TCH, TC)
    xTh = np.ascontiguousarray(xTh.transpose(0, 3, 1, 2, 4)).astype(NPBF16)
    xflat = xTh.reshape(-1)

    in_maps = []
    for c in range(N_CORES):
        sl = slice(c * HPC, (c + 1) * HPC)
        blob = np.concatenate([
            xflat,
            np.ascontiguousarray(Wq[sl].T).astype(NPBF16).reshape(-1),
            np.ascontiguousarray(Wk[sl].T).astype(NPBF16).reshape(-1),
            np.ascontiguousarray(Wv[sl].T).astype(NPBF16).reshape(-1),
            np.ascontiguousarray(Wo[:, sl].T).astype(NPBF16).reshape(-1),
        ])
        assert blob.shape == (BLOB_SZ,)
        m = {"blob": blob}
        if use_bias:
            m["bq"] = np.ascontiguousarray(bq[sl]).reshape(HEADS_PC, 128)
            m["bk"] = np.ascontiguousarray(bk[sl]).reshape(HEADS_PC, 128)
            m["bv"] = np.ascontiguousarray(bv[sl]).reshape(1, HPC)
        if use_mask:
            mt = mask[:, 0].transpose(0, 2, 1) / SCALE  # [B, tk, tq]
            m["maskT"] = np.ascontiguousarray(mt).reshape(B, KT, 128, T)
        in_maps.append(m)
    return nc, in_maps


def postprocess(results, inputs):
    bo = np.asarray(inputs["bo"], dtype=np.float32)
    acc = results[0]["out"].astype(np.float32)
    for c in range(1, N_CORES):
        acc = acc + results[c]["out"].astype(np.float32)
    return (acc + bo).reshape(B, T, H)


def kernel(**inputs):
    import time as _time

    nc, in_maps = prepare(inputs)
    last_err = None
    for attempt in range(3):
        try:
            res = run_bass_kernel_spmd(nc, in_maps, list(range(N_CORES)))
            result = postprocess(res.results, inputs)
            if np.isfinite(result).all():
                return result
            # transient device fault can yield garbage without raising
            last_err = ValueError("non-finite kernel output")
        except Exception as e:
            last_err = e
        _time.sleep(2.0)
        try:  # best-effort device recovery before retrying
            import jax
            jax.extend.backend.clear_backends()
        except Exception:
            pass
    raise last_err


# revision 15
# speedup vs baseline: 1.4947x; 1.0169x over previous
"""GroupQueryAttention (16 heads, hd=128) on 8 trn2 cores, heads sharded 2/core.

v2: bf16 data path. x, Wq/Wk/Wv/Wo, q, k, v, probs, attnT all bf16 in
SBUF/DMA; every matmul accumulates in fp32 PSUM. Halves HBM traffic and
DVE element counts vs f32, and enables Fast Weight Load on the PE
(fp32 weights cannot FWL).

Layouts (per core c, host-prepped):
  xT    [B, 4, 16, 128, 512] bf16  x[b].T chunked: (chunk, ktile, h-part, t-col)
  wqT/wkT/wvT [16, 128, 256] bf16  W[256c:256c+256,:].T chunked by h-ktile
  woT   [2, 128, 2048] bf16        Wo[:, 256c:256c+256].T per local head
  out   [4096, 2048] f32           partial product, host sums over cores

Device per (b, h): scoresT[tk,tq] = kT.T@qT -> exp (ACT, psum->sbuf bf16) ->
PV attnT[hd,tq] = v.T-chain; rowsum via ones-col matmul over a DVE
bf16 add-tree of the 16 prob tiles; normalize attnT via PE-broadcast
reciprocal; out-proj from attnT (bf16 stationary) @ woT.
"""
import sys

for _p in ("/opt/trn_rl_repo",):
    if _p not in sys.path:
        sys.path.insert(0, _p)

import numpy as np
import ml_dtypes

import concourse.bass as bass
import concourse.tile as tile
from concourse import bacc, mybir
from concourse.bass_utils import run_bass_kernel_spmd

N_CORES = 8
B, T, H = 2, 2048, 2048
NH, HD = 16, 128
HPC = H // N_CORES          # 256 dims (2 heads) per core
HEADS_PC = NH // N_CORES    # 2
KT = H // 128               # 16 k-tiles along hidden
TCH = 4                     # t-chunks (512 cols) per batch for projections
TC = T // TCH               # 512
QC = 512                    # tq chunk in attention
NQC = T // QC               # 4
SCALE = float(HD) ** -0.5
XT_SZ = B * TCH * KT * 128 * TC      # hidden_states.T, chunked
W_SZ = KT * 128 * HPC                # one projection weight slice
BLOB_SZ = XT_SZ + 4 * W_SZ

F32 = mybir.dt.float32
F32R = mybir.dt.float32r
BF16 = mybir.dt.bfloat16
AF = mybir.ActivationFunctionType
OP = mybir.AluOpType
NPBF16 = ml_dtypes.bfloat16


def r(ap):
    return ap


_CACHE = {}


def _build(use_mask, use_bias):
    key = (use_mask, use_bias)
    if key in _CACHE:
        return _CACHE[key]

    nc = bacc.Bacc("TRN2", target_bir_lowering=False, debug=False,
                   num_devices=N_CORES)
    # all inputs packed into one blob: per-call dispatch cost through the
    # axon tunnel scales with operand count (~37us/operand), so one buffer
    # beats six
    blob = nc.dram_tensor("blob", [BLOB_SZ], BF16, kind="ExternalInput").ap()
    xT = blob[0:XT_SZ].rearrange("(b c i p j) -> b c i p j",
                                 b=B, c=TCH, i=KT, p=128, j=TC)
    _o = XT_SZ
    wqT = blob[_o:_o + W_SZ].rearrange("(i p j) -> i p j", i=KT, p=128, j=HPC)
    _o += W_SZ
    wkT = blob[_o:_o + W_SZ].rearrange("(i p j) -> i p j", i=KT, p=128, j=HPC)
    _o += W_SZ
    wvT = blob[_o:_o + W_SZ].rearrange("(i p j) -> i p j", i=KT, p=128, j=HPC)
    _o += W_SZ
    woT = blob[_o:_o + W_SZ].rearrange("(h p j) -> h p j", h=HEADS_PC, p=128, j=H)
    if use_bias:
        bqd = nc.dram_tensor("bq", [HEADS_PC, 128], F32, kind="ExternalInput").ap()
        bkd = nc.dram_tensor("bk", [HEADS_PC, 128], F32, kind="ExternalInput").ap()
        bvd = nc.dram_tensor("bv", [1, HPC], F32R, kind="ExternalInput").ap()
    if use_mask:
        # mask[b,0].T / SCALE, tk-tiled
        mkd = nc.dram_tensor("maskT", [B, KT, 128, T], F32, kind="ExternalInput").ap()
    out = nc.dram_tensor("out", [B * T, H], BF16, kind="ExternalOutput").ap()

    from contextlib import ExitStack
    with tile.TileContext(nc) as tc, ExitStack() as ctx:
        wpool = ctx.enter_context(tc.tile_pool(name="wts", bufs=1))
        cpool = ctx.enter_context(tc.tile_pool(name="consts", bufs=1))
        xpool = ctx.enter_context(tc.tile_pool(name="xt", bufs=2))
        qkv_pool = ctx.enter_context(tc.tile_pool(name="qkv", bufs=1))
        pr_pool = ctx.enter_context(tc.tile_pool(name="probs", bufs=4))
        acc_pool = ctx.enter_context(tc.tile_pool(name="acc", bufs=2))
        rec_pool = ctx.enter_context(tc.tile_pool(name="rec", bufs=2))
        bcs_pool = ctx.enter_context(tc.tile_pool(name="bcs", bufs=2))
        at_pool = ctx.enter_context(tc.tile_pool(name="attnT", bufs=1))
        os_pool = ctx.enter_context(tc.tile_pool(name="osb", bufs=3))
        if use_mask:
            mk_pool = ctx.enter_context(tc.tile_pool(name="mask", bufs=4))

        proj_ps = ctx.enter_context(tc.tile_pool(name="proj_ps", bufs=2, space="PSUM"))
        sc_ps = ctx.enter_context(tc.tile_pool(name="sc_ps", bufs=2, space="PSUM"))
        pv_ps = ctx.enter_context(tc.tile_pool(name="pv_ps", bufs=2, space="PSUM"))
        rb_ps = ctx.enter_context(tc.tile_pool(name="rb_ps", bufs=1, space="PSUM"))

        # ---- load weights / constants ----
        def load_w(dram, tag):
            t = wpool.tile([128, KT * HPC], BF16, tag=tag)
            nc.sync.dma_start(t[:].rearrange("p (i j) -> p i j", j=HPC),
                              dram.rearrange("i p j -> p i j"))
            return t

        xt00 = xpool.tile([128, KT * TC], BF16, tag="xt", name="xt00")
        wq = wpool.tile([128, KT * HPC], BF16, tag="wqT", name="wq_t")
        wk = wpool.tile([128, KT * HPC], BF16, tag="wk", name="wk_t")
        for qtr in range(4):
            ksl = slice(4 * qtr, 4 * (qtr + 1))
            nc.sync.dma_start(
                xt00[:, 4 * qtr * TC:4 * (qtr + 1) * TC].rearrange(
                    "p (i j) -> p i j", j=TC),
                xT[0, 0, ksl].rearrange("i p j -> p i j"))
            for t_, d_ in ((wq, wqT), (wk, wkT)):
                nc.sync.dma_start(
                    t_[:, 4 * qtr * HPC:4 * (qtr + 1) * HPC].rearrange(
                        "p (i j) -> p i j", j=HPC),
                    d_[ksl].rearrange("i p j -> p i j"))
        wv = load_w(wvT, "wv")
        wo = wpool.tile([128, HEADS_PC * H], BF16, tag="wo")

        ones_col = cpool.tile([128, 1], BF16, tag="ones_col")
        nc.vector.memset(ones_col[:], 1.0)
        # memset can't write f32r directly (ISA reject): stage via f32
        ones_row_f = cpool.tile([1, 128], F32, tag="ones_row_f")
        nc.vector.memset(ones_row_f[:], 1.0)
        ones_row = cpool.tile([1, 128], F32R, tag="ones_row")
        nc.vector.tensor_copy(ones_row[:], ones_row_f[:])

        if use_bias:
            bq_t = cpool.tile([128, HEADS_PC], F32, tag="bq")
            nc.sync.dma_start(bq_t[:], bqd.rearrange("h p -> p h"))
            bk_t = cpool.tile([128, HEADS_PC], F32, tag="bk")
            nc.sync.dma_start(bk_t[:], bkd.rearrange("h p -> p h"))
            bv_row = cpool.tile([1, HPC], F32R, tag="bv_row")
            nc.sync.dma_start(bv_row[:], bvd)
            bv_ps = rb_ps.tile([128, HPC], F32, tag="rb")
            nc.tensor.matmul(bv_ps[:], r(ones_row[:]), r(bv_row[:]),
                             start=True, stop=True)
            bv_bc = cpool.tile([128, HPC], F32, tag="bv_bc")
            nc.vector.tensor_copy(bv_bc[:], bv_ps[:])

        for b in range(B):
            # ---- q/k/v projections for this batch ----
            qT = [qkv_pool.tile([128, T], BF16, tag=f"q{h}", name=f"qT{h}", bufs=2)
                  for h in range(HEADS_PC)]
            kTt = [qkv_pool.tile([128, T], BF16, tag=f"k{h}", name=f"kT{h}", bufs=2)
                   for h in range(HEADS_PC)]
            vt = qkv_pool.tile([128, KT * HPC], BF16, tag="v")  # [t-tile, d]

            for c in range(TCH):
                if b == 0 and c == 0:
                    xt = xt00
                else:
                    xt = xpool.tile([128, KT * TC], BF16, tag="xt")
                    nc.sync.dma_start(xt[:].rearrange("p (i j) -> p i j", j=TC),
                                      xT[b, c].rearrange("i p j -> p i j"))
                fastpath0 = b == 0 and c == 0 and not use_bias
                if fastpath0:
                    ps_q = proj_ps.tile([128, TC], F32, tag="proj", name="ps_q0")
                    ps_k = proj_ps.tile([128, TC], F32, tag="proj", name="ps_k0")
                    for i in range(KT):
                        for w_, ps_ in ((wq, ps_q), (wk, ps_k)):
                            nc.tensor.matmul(
                                ps_[:],
                                r(w_[:, i * HPC: i * HPC + 128]),
                                r(xt[:, i * TC: (i + 1) * TC]),
                                start=(i == 0), stop=(i == KT - 1))
                    nc.vector.tensor_copy(qT[0][:, 0:TC], ps_q[:])
                    nc.vector.tensor_copy(kTt[0][:, 0:TC], ps_k[:])
                heads_todo = [1] if fastpath0 else list(range(HEADS_PC))
                for h in heads_todo:
                    for w_, dst, bias_t in ((wq, qT[h], "bq"), (wk, kTt[h], "bk")):
                        ps = proj_ps.tile([128, TC], F32, tag="proj")
                        for i in range(KT):
                            nc.tensor.matmul(
                                ps[:],
                                r(w_[:, i * HPC + 128 * h: i * HPC + 128 * h + 128]),
                                r(xt[:, i * TC: (i + 1) * TC]),
                                start=(i == 0), stop=(i == KT - 1))
                        if use_bias:
                            bt = bq_t if bias_t == "bq" else bk_t
                            nc.scalar.activation(dst[:, c * TC:(c + 1) * TC], ps[:],
                                                 AF.Identity, bias=bt[:, h:h + 1])
                        else:
                            nc.vector.tensor_copy(dst[:, c * TC:(c + 1) * TC], ps[:])
                for s in range(4):  # four 128-row t-subtiles of this chunk
                    tt = 4 * c + s
                    ps = proj_ps.tile([128, HPC], F32, tag="proj")
                    for i in range(KT):
                        nc.tensor.matmul(
                            ps[:],
                            r(xt[:, i * TC + 128 * s: i * TC + 128 * s + 128]),
                            r(wv[:, i * HPC: (i + 1) * HPC]),
                            start=(i == 0), stop=(i == KT - 1))
                    if use_bias:
                        nc.vector.scalar_tensor_tensor(
                            vt[:, tt * HPC:(tt + 1) * HPC], ps[:], 1.0, bv_bc[:],
                            op0=OP.mult, op1=OP.add)
                    else:
                        nc.vector.tensor_copy(vt[:, tt * HPC:(tt + 1) * HPC], ps[:])

            if b == 0:
                nc.sync.dma_start(wo[:].rearrange("p (i j) -> p i j", j=H),
                                  woT.rearrange("i p j -> p i j"))

            # ---- attention (chunk-outer) interleaved with out-proj ----
            for ch in range(NQC):
                attnT = [at_pool.tile([128, QC], BF16, tag=f"a{h}", name=f"attnT{h}", bufs=2)
                         for h in range(HEADS_PC)]
                for h in range(HEADS_PC):
                    q_sl = r(qT[h][:, ch * QC:(ch + 1) * QC])
                    pv = pv_ps.tile([128, QC], F32, tag="pv")
                    rs = rb_ps.tile([1, QC], F32, tag="rb")
                    prs = []
                    lvl = {}  # add-tree: level -> pending tile
                    for i in range(KT):
                        sc = sc_ps.tile([128, QC], F32, tag="sc")
                        nc.tensor.matmul(sc[:], r(kTt[h][:, i * 128:(i + 1) * 128]),
                                         q_sl, start=True, stop=True)
                        if use_mask:
                            mk = mk_pool.tile([128, QC], F32, tag="mk")
                            nc.sync.dma_start(mk[:], mkd[b, i, :, ch * QC:(ch + 1) * QC])
                            nc.vector.tensor_add(sc[:], sc[:], mk[:])
                        pr = pr_pool.tile([128, QC], BF16, tag="pr")
                        nc.scalar.activation(pr[:], sc[:], AF.Exp, scale=SCALE)
                        nc.tensor.matmul(
                            pv[:],
                            r(vt[:, i * HPC + 128 * h: i * HPC + 128 * h + 128]),
                            r(pr[:]), start=(i == 0), stop=(i == KT - 1))
                        prs.append(pr)
                        # binary add-tree over the 16 prob tiles (DVE, bf16)
                        node, l = pr, 0
                        while l in lvl:
                            nxt = acc_pool.tile([128, QC], BF16, tag=f"t{l}",
                                                name=f"tree{l}")
                            nc.vector.tensor_add(nxt[:], lvl.pop(l)[:], node[:])
                            node, l = nxt, l + 1
                        lvl[l] = node
                    total = lvl[4]  # 16 tiles -> single level-4 node
                    nc.tensor.matmul(rs[:], r(ones_col[:]), r(total[:]),
                                     start=True, stop=True)
                    rec = rec_pool.tile([1, QC], F32R, tag="rec")
                    with nc.allow_low_precision(reason="f32r rowsum reciprocal"):
                        nc.vector.reciprocal(rec[:], rs[:])
                    bc = rb_ps.tile([128, QC], F32, tag="rb")
                    nc.tensor.matmul(bc[:], r(ones_row[:]), r(rec[:]),
                                     start=True, stop=True)
                    bcs = bcs_pool.tile([128, QC], F32, tag="bcs")
                    nc.vector.tensor_copy(bcs[:], bc[:])
                    nc.vector.scalar_tensor_tensor(
                        attnT[h][:], pv[:], 1.0, bcs[:],
                        op0=OP.mult, op1=OP.mult)

                # out-proj for this chunk's 4 t-tiles (partial over local dims)
                for st in range(4):
                    tt = 4 * ch + st
                    for oc in range(4):
                        ps = proj_ps.tile([128, 512], F32, tag="oproj", bufs=1)
                        for h in range(HEADS_PC):
                            nc.tensor.matmul(
                                ps[:],
                                r(attnT[h][:, st * 128:(st + 1) * 128]),
                                r(wo[:, h * H + oc * 512: h * H + (oc + 1) * 512]),
                                start=(h == 0), stop=(h == HEADS_PC - 1))
                        ob = os_pool.tile([128, 512], BF16, tag="ob")
                        if (st + oc) % 2 == 0:
                            nc.vector.tensor_copy(ob[:], ps[:])
                        else:
                            nc.scalar.copy(ob[:], ps[:])
                        nc.gpsimd.dma_start(
                            out[b * T + tt * 128: b * T + (tt + 1) * 128,
                                oc * 512:(oc + 1) * 512], ob[:])


    nc.compile()
    _CACHE[key] = nc
    return nc


def prepare(inputs):
    hs = np.ascontiguousarray(np.asarray(inputs["hidden_states"], dtype=np.float32))
    mask = np.asarray(inputs["attention_mask"], dtype=np.float32)
    Wq = np.asarray(inputs["Wq"], dtype=np.float32)
    Wk = np.asarray(inputs["Wk"], dtype=np.float32)
    Wv = np.asarray(inputs["Wv"], dtype=np.float32)
    Wo = np.asarray(inputs["Wo"], dtype=np.float32)
    bq = np.asarray(inputs["bq"], dtype=np.float32)
    bk = np.asarray(inputs["bk"], dtype=np.float32)
    bv = np.asarray(inputs["bv"], dtype=np.float32)

    use_mask = bool(np.any(mask))
    use_bias = bool(np.any(bq) or np.any(bk) or np.any(bv))
    nc = _build(use_mask, use_bias)

    # x[b].T -> [h,t] -> (16,128, 4,512) -> [4,16,128,512]
    xTh = hs.transpose(0, 2, 1).reshape(B, KT, 128, TCH, TC)
    xTh = np.ascontiguousarray(xTh.transpose(0, 3, 1, 2, 4)).astype(NPBF16)
    xflat = xTh.reshape(-1)

    in_maps = []
    for c in range(N_CORES):
        sl = slice(c * HPC, (c + 1) * HPC)
        blob = np.concatenate([
            xflat,
            np.ascontiguousarray(Wq[sl].T).astype(NPBF16).reshape(-1),
            np.ascontiguousarray(Wk[sl].T).astype(NPBF16).reshape(-1),
            np.ascontiguousarray(Wv[sl].T).astype(NPBF16).reshape(-1),
            np.ascontiguousarray(Wo[:, sl].T).astype(NPBF16).reshape(-1),
        ])
        assert blob.shape == (BLOB_SZ,)
        m = {"blob": blob}
        if use_bias:
            m["bq"] = np.ascontiguousarray(bq[sl]).reshape(HEADS_PC, 128)
            m["bk"] = np.ascontiguousarray(bk[sl]).reshape(HEADS_PC, 128)
            m["bv"] = np.ascontiguousarray(bv[sl]).reshape(1, HPC)
        if use_mask:
            mt = mask[:, 0].transpose(0, 2, 1) / SCALE  # [B, tk, tq]
            m["maskT"] = np.ascontiguousarray(mt).reshape(B, KT, 128, T)
        in_maps.append(m)
    return nc, in_maps


def postprocess(results, inputs):
    bo = np.asarray(inputs["bo"], dtype=np.float32)
    acc = results[0]["out"].astype(np.float32)
    for c in range(1, N_CORES):
        acc = acc + results[c]["out"].astype(np.float32)
    return (acc + bo).reshape(B, T, H)


def kernel(**inputs):
    import time as _time

    nc, in_maps = prepare(inputs)
    last_err = None
    for attempt in range(3):
        try:
            res = run_bass_kernel_spmd(nc, in_maps, list(range(N_CORES)))
            result = postprocess(res.results, inputs)
            if np.isfinite(result).all():
                return result
            # transient device fault can yield garbage without raising
            last_err = ValueError("non-finite kernel output")
        except Exception as e:
            last_err = e
        _time.sleep(2.0)
        try:  # best-effort device recovery before retrying
            import jax
            jax.extend.backend.clear_backends()
        except Exception:
            pass
    raise last_err


# revision 16
# speedup vs baseline: 1.5039x; 1.0062x over previous
"""GroupQueryAttention (16 heads, hd=128) on 8 trn2 cores, heads sharded 2/core.

v2: bf16 data path. x, Wq/Wk/Wv/Wo, q, k, v, probs, attnT all bf16 in
SBUF/DMA; every matmul accumulates in fp32 PSUM. Halves HBM traffic and
DVE element counts vs f32, and enables Fast Weight Load on the PE
(fp32 weights cannot FWL).

Layouts (per core c, host-prepped):
  xT    [B, 4, 16, 128, 512] bf16  x[b].T chunked: (chunk, ktile, h-part, t-col)
  wqT/wkT/wvT [16, 128, 256] bf16  W[256c:256c+256,:].T chunked by h-ktile
  woT   [2, 128, 2048] bf16        Wo[:, 256c:256c+256].T per local head
  out   [4096, 2048] f32           partial product, host sums over cores

Device per (b, h): scoresT[tk,tq] = kT.T@qT -> exp (ACT, psum->sbuf bf16) ->
PV attnT[hd,tq] = v.T-chain; rowsum via ones-col matmul over a DVE
bf16 add-tree of the 16 prob tiles; normalize attnT via PE-broadcast
reciprocal; out-proj from attnT (bf16 stationary) @ woT.
"""
import sys

for _p in ("/opt/trn_rl_repo",):
    if _p not in sys.path:
        sys.path.insert(0, _p)

import numpy as np
import ml_dtypes

import concourse.bass as bass
import concourse.tile as tile
from concourse import bacc, mybir
from concourse.bass_utils import run_bass_kernel_spmd

N_CORES = 8
B, T, H = 2, 2048, 2048
NH, HD = 16, 128
HPC = H // N_CORES          # 256 dims (2 heads) per core
HEADS_PC = NH // N_CORES    # 2
KT = H // 128               # 16 k-tiles along hidden
TCH = 4                     # t-chunks (512 cols) per batch for projections
TC = T // TCH               # 512
QC = 512                    # tq chunk in attention
NQC = T // QC               # 4
SCALE = float(HD) ** -0.5
XT_SZ = B * TCH * KT * 128 * TC      # hidden_states.T, chunked
W_SZ = KT * 128 * HPC                # one projection weight slice
BLOB_SZ = XT_SZ + 4 * W_SZ

F32 = mybir.dt.float32
F32R = mybir.dt.float32r
BF16 = mybir.dt.bfloat16
AF = mybir.ActivationFunctionType
OP = mybir.AluOpType
NPBF16 = ml_dtypes.bfloat16


def r(ap):
    return ap


_CACHE = {}


def _build(use_mask, use_bias):
    key = (use_mask, use_bias)
    if key in _CACHE:
        return _CACHE[key]

    nc = bacc.Bacc("TRN2", target_bir_lowering=False, debug=False,
                   num_devices=N_CORES)
    # all inputs packed into one blob: per-call dispatch cost through the
    # axon tunnel scales with operand count (~37us/operand), so one buffer
    # beats six
    blob = nc.dram_tensor("blob", [BLOB_SZ], BF16, kind="ExternalInput").ap()
    xT = blob[0:XT_SZ].rearrange("(b c i p j) -> b c i p j",
                                 b=B, c=TCH, i=KT, p=128, j=TC)
    _o = XT_SZ
    wqT = blob[_o:_o + W_SZ].rearrange("(i p j) -> i p j", i=KT, p=128, j=HPC)
    _o += W_SZ
    wkT = blob[_o:_o + W_SZ].rearrange("(i p j) -> i p j", i=KT, p=128, j=HPC)
    _o += W_SZ
    wvT = blob[_o:_o + W_SZ].rearrange("(i p j) -> i p j", i=KT, p=128, j=HPC)
    _o += W_SZ
    woT = blob[_o:_o + W_SZ].rearrange("(h p j) -> h p j", h=HEADS_PC, p=128, j=H)
    if use_bias:
        bqd = nc.dram_tensor("bq", [HEADS_PC, 128], F32, kind="ExternalInput").ap()
        bkd = nc.dram_tensor("bk", [HEADS_PC, 128], F32, kind="ExternalInput").ap()
        bvd = nc.dram_tensor("bv", [1, HPC], F32R, kind="ExternalInput").ap()
    if use_mask:
        # mask[b,0].T / SCALE, tk-tiled
        mkd = nc.dram_tensor("maskT", [B, KT, 128, T], F32, kind="ExternalInput").ap()
    out = nc.dram_tensor("out", [B * T, H], BF16, kind="ExternalOutput").ap()

    from contextlib import ExitStack
    with tile.TileContext(nc) as tc, ExitStack() as ctx:
        wpool = ctx.enter_context(tc.tile_pool(name="wts", bufs=1))
        cpool = ctx.enter_context(tc.tile_pool(name="consts", bufs=1))
        xpool = ctx.enter_context(tc.tile_pool(name="xt", bufs=2))
        qkv_pool = ctx.enter_context(tc.tile_pool(name="qkv", bufs=1))
        pr_pool = ctx.enter_context(tc.tile_pool(name="probs", bufs=4))
        acc_pool = ctx.enter_context(tc.tile_pool(name="acc", bufs=2))
        rec_pool = ctx.enter_context(tc.tile_pool(name="rec", bufs=2))
        bcs_pool = ctx.enter_context(tc.tile_pool(name="bcs", bufs=2))
        at_pool = ctx.enter_context(tc.tile_pool(name="attnT", bufs=1))
        os_pool = ctx.enter_context(tc.tile_pool(name="osb", bufs=3))
        if use_mask:
            mk_pool = ctx.enter_context(tc.tile_pool(name="mask", bufs=4))

        proj_ps = ctx.enter_context(tc.tile_pool(name="proj_ps", bufs=2, space="PSUM"))
        sc_ps = ctx.enter_context(tc.tile_pool(name="sc_ps", bufs=2, space="PSUM"))
        pv_ps = ctx.enter_context(tc.tile_pool(name="pv_ps", bufs=2, space="PSUM"))
        rb_ps = ctx.enter_context(tc.tile_pool(name="rb_ps", bufs=1, space="PSUM"))

        # ---- load weights / constants ----
        def load_w(dram, tag):
            t = wpool.tile([128, KT * HPC], BF16, tag=tag)
            nc.sync.dma_start(t[:].rearrange("p (i j) -> p i j", j=HPC),
                              dram.rearrange("i p j -> p i j"))
            return t

        xt00 = xpool.tile([128, KT * TC], BF16, tag="xt", name="xt00")
        wq = wpool.tile([128, KT * HPC], BF16, tag="wqT", name="wq_t")
        wk = wpool.tile([128, KT * HPC], BF16, tag="wk", name="wk_t")
        for qtr in range(4):
            ksl = slice(4 * qtr, 4 * (qtr + 1))
            nc.sync.dma_start(
                xt00[:, 4 * qtr * TC:4 * (qtr + 1) * TC].rearrange(
                    "p (i j) -> p i j", j=TC),
                xT[0, 0, ksl].rearrange("i p j -> p i j"))
            for t_, d_ in ((wq, wqT), (wk, wkT)):
                nc.sync.dma_start(
                    t_[:, 4 * qtr * HPC:4 * (qtr + 1) * HPC].rearrange(
                        "p (i j) -> p i j", j=HPC),
                    d_[ksl].rearrange("i p j -> p i j"))
        wv = load_w(wvT, "wv")
        wo = wpool.tile([128, HEADS_PC * H], BF16, tag="wo")

        ones_col = cpool.tile([128, 1], BF16, tag="ones_col")
        nc.vector.memset(ones_col[:], 1.0)
        # memset can't write f32r directly (ISA reject): stage via f32
        ones_row_f = cpool.tile([1, 128], F32, tag="ones_row_f")
        nc.vector.memset(ones_row_f[:], 1.0)
        ones_row = cpool.tile([1, 128], F32R, tag="ones_row")
        nc.vector.tensor_copy(ones_row[:], ones_row_f[:])

        if use_bias:
            bq_t = cpool.tile([128, HEADS_PC], F32, tag="bq")
            nc.sync.dma_start(bq_t[:], bqd.rearrange("h p -> p h"))
            bk_t = cpool.tile([128, HEADS_PC], F32, tag="bk")
            nc.sync.dma_start(bk_t[:], bkd.rearrange("h p -> p h"))
            bv_row = cpool.tile([1, HPC], F32R, tag="bv_row")
            nc.sync.dma_start(bv_row[:], bvd)
            bv_ps = rb_ps.tile([128, HPC], F32, tag="rb")
            nc.tensor.matmul(bv_ps[:], r(ones_row[:]), r(bv_row[:]),
                             start=True, stop=True)
            bv_bc = cpool.tile([128, HPC], F32, tag="bv_bc")
            nc.vector.tensor_copy(bv_bc[:], bv_ps[:])

        for b in range(B):
            # ---- q/k/v projections for this batch ----
            qT = [qkv_pool.tile([128, T], BF16, tag=f"q{h}", name=f"qT{h}", bufs=2)
                  for h in range(HEADS_PC)]
            kTt = [qkv_pool.tile([128, T], BF16, tag=f"k{h}", name=f"kT{h}", bufs=2)
                   for h in range(HEADS_PC)]
            vt = qkv_pool.tile([128, KT * HPC], BF16, tag="v")  # [t-tile, d]

            for c in range(TCH):
                if b == 0 and c == 0:
                    xt = xt00
                else:
                    xt = xpool.tile([128, KT * TC], BF16, tag="xt")
                    nc.sync.dma_start(xt[:].rearrange("p (i j) -> p i j", j=TC),
                                      xT[b, c].rearrange("i p j -> p i j"))
                fastpath0 = b == 0 and c == 0 and not use_bias
                if fastpath0:
                    ps_q = proj_ps.tile([128, TC], F32, tag="proj", name="ps_q0")
                    ps_k = proj_ps.tile([128, TC], F32, tag="proj", name="ps_k0")
                    for i in range(KT):
                        for w_, ps_ in ((wq, ps_q), (wk, ps_k)):
                            nc.tensor.matmul(
                                ps_[:],
                                r(w_[:, i * HPC: i * HPC + 128]),
                                r(xt[:, i * TC: (i + 1) * TC]),
                                start=(i == 0), stop=(i == KT - 1))
                    nc.vector.tensor_copy(qT[0][:, 0:TC], ps_q[:])
                    nc.vector.tensor_copy(kTt[0][:, 0:TC], ps_k[:])
                heads_todo = [1] if fastpath0 else list(range(HEADS_PC))
                for h in heads_todo:
                    for w_, dst, bias_t in ((wq, qT[h], "bq"), (wk, kTt[h], "bk")):
                        ps = proj_ps.tile([128, TC], F32, tag="proj")
                        for i in range(KT):
                            nc.tensor.matmul(
                                ps[:],
                                r(w_[:, i * HPC + 128 * h: i * HPC + 128 * h + 128]),
                                r(xt[:, i * TC: (i + 1) * TC]),
                                start=(i == 0), stop=(i == KT - 1))
                        if use_bias:
                            bt = bq_t if bias_t == "bq" else bk_t
                            nc.scalar.activation(dst[:, c * TC:(c + 1) * TC], ps[:],
                                                 AF.Identity, bias=bt[:, h:h + 1])
                        else:
                            nc.vector.tensor_copy(dst[:, c * TC:(c + 1) * TC], ps[:])
                for s in range(4):  # four 128-row t-subtiles of this chunk
                    tt = 4 * c + s
                    ps = proj_ps.tile([128, HPC], F32, tag="proj")
                    for i in range(KT):
                        nc.tensor.matmul(
                            ps[:],
                            r(xt[:, i * TC + 128 * s: i * TC + 128 * s + 128]),
                            r(wv[:, i * HPC: (i + 1) * HPC]),
                            start=(i == 0), stop=(i == KT - 1))
                    if use_bias:
                        nc.vector.scalar_tensor_tensor(
                            vt[:, tt * HPC:(tt + 1) * HPC], ps[:], 1.0, bv_bc[:],
                            op0=OP.mult, op1=OP.add)
                    else:
                        nc.vector.tensor_copy(vt[:, tt * HPC:(tt + 1) * HPC], ps[:])

            if b == 0:
                nc.sync.dma_start(wo[:].rearrange("p (i j) -> p i j", j=H),
                                  woT.rearrange("i p j -> p i j"))

            # ---- attention (chunk-outer) interleaved with out-proj ----
            for ch in range(NQC):
                attnT = [at_pool.tile([128, QC], BF16, tag=f"a{h}", name=f"attnT{h}", bufs=2)
                         for h in range(HEADS_PC)]
                for h in range(HEADS_PC):
                    q_sl = r(qT[h][:, ch * QC:(ch + 1) * QC])
                    pv = pv_ps.tile([128, QC], F32, tag="pv")
                    rs = rb_ps.tile([1, QC], F32, tag="rb")
                    prs = []
                    lvl = {}  # add-tree: level -> pending tile
                    for i in range(KT):
                        sc = sc_ps.tile([128, QC], F32, tag="sc")
                        nc.tensor.matmul(sc[:], r(kTt[h][:, i * 128:(i + 1) * 128]),
                                         q_sl, start=True, stop=True)
                        if use_mask:
                            mk = mk_pool.tile([128, QC], F32, tag="mk")
                            nc.sync.dma_start(mk[:], mkd[b, i, :, ch * QC:(ch + 1) * QC])
                            nc.vector.tensor_add(sc[:], sc[:], mk[:])
                        pr = pr_pool.tile([128, QC], BF16, tag="pr")
                        nc.scalar.activation(pr[:], sc[:], AF.Exp, scale=SCALE)
                        nc.tensor.matmul(
                            pv[:],
                            r(vt[:, i * HPC + 128 * h: i * HPC + 128 * h + 128]),
                            r(pr[:]), start=(i == 0), stop=(i == KT - 1))
                        prs.append(pr)
                        # binary add-tree over the 16 prob tiles (DVE, bf16)
                        node, l = pr, 0
                        while l in lvl:
                            nxt = acc_pool.tile([128, QC], BF16, tag=f"t{l}",
                                                name=f"tree{l}")
                            nc.vector.tensor_add(nxt[:], lvl.pop(l)[:], node[:])
                            node, l = nxt, l + 1
                        lvl[l] = node
                    total = lvl[4]  # 16 tiles -> single level-4 node
                    nc.tensor.matmul(rs[:], r(ones_col[:]), r(total[:]),
                                     start=True, stop=True)
                    rec = rec_pool.tile([1, QC], F32R, tag="rec")
                    with nc.allow_low_precision(reason="f32r rowsum reciprocal"):
                        nc.vector.reciprocal(rec[:], rs[:])
                    bc = rb_ps.tile([128, QC], F32, tag="rb")
                    nc.tensor.matmul(bc[:], r(ones_row[:]), r(rec[:]),
                                     start=True, stop=True)
                    bcs = bcs_pool.tile([128, QC], F32, tag="bcs")
                    nc.vector.tensor_copy(bcs[:], bc[:])
                    nc.vector.scalar_tensor_tensor(
                        attnT[h][:], pv[:], 1.0, bcs[:],
                        op0=OP.mult, op1=OP.mult)

                # out-proj for this chunk's 4 t-tiles (partial over local dims)
                final_chunk = b == B - 1 and ch == NQC - 1
                for st in range(4):
                    tt = 4 * ch + st
                    for oc in range(4):
                        # the final chunk's blocks have no later proj-phase to
                        # WAR-couple with, so alternate them across two psum
                        # tags to overlap the staging copies
                        if final_chunk and oc % 2 == 1:
                            ps = proj_ps.tile([128, 512], F32, tag="proj")
                        else:
                            ps = proj_ps.tile([128, 512], F32, tag="oproj",
                                              bufs=1)
                        for h in range(HEADS_PC):
                            nc.tensor.matmul(
                                ps[:],
                                r(attnT[h][:, st * 128:(st + 1) * 128]),
                                r(wo[:, h * H + oc * 512: h * H + (oc + 1) * 512]),
                                start=(h == 0), stop=(h == HEADS_PC - 1))
                        ob = os_pool.tile([128, 512], BF16, tag="ob")
                        if (st + oc) % 2 == 0:
                            nc.vector.tensor_copy(ob[:], ps[:])
                        else:
                            nc.scalar.copy(ob[:], ps[:])
                        nc.gpsimd.dma_start(
                            out[b * T + tt * 128: b * T + (tt + 1) * 128,
                                oc * 512:(oc + 1) * 512], ob[:])


    nc.compile()
    _CACHE[key] = nc
    return nc


def prepare(inputs):
    hs = np.ascontiguousarray(np.asarray(inputs["hidden_states"], dtype=np.float32))
    mask = np.asarray(inputs["attention_mask"], dtype=np.float32)
    Wq = np.asarray(inputs["Wq"], dtype=np.float32)
    Wk = np.asarray(inputs["Wk"], dtype=np.float32)
    Wv = np.asarray(inputs["Wv"], dtype=np.float32)
    Wo = np.asarray(inputs["Wo"], dtype=np.float32)
    bq = np.asarray(inputs["bq"], dtype=np.float32)
    bk = np.asarray(inputs["bk"], dtype=np.float32)
    bv = np.asarray(inputs["bv"], dtype=np.float32)

    use_mask = bool(np.any(mask))
    use_bias = bool(np.any(bq) or np.any(bk) or np.any(bv))
    nc = _build(use_mask, use_bias)

    # x[b].T -> [h,t] -> (16,128, 4,512) -> [4,16,128,512]
    xTh = hs.transpose(0, 2, 1).reshape(B, KT, 128, TCH, TC)
    xTh = np.ascontiguousarray(xTh.transpose(0, 3, 1, 2, 4)).astype(NPBF16)
    xflat = xTh.reshape(-1)

    in_maps = []
    for c in range(N_CORES):
        sl = slice(c * HPC, (c + 1) * HPC)
        blob = np.concatenate([
            xflat,
            np.ascontiguousarray(Wq[sl].T).astype(NPBF16).reshape(-1),
            np.ascontiguousarray(Wk[sl].T).astype(NPBF16).reshape(-1),
            np.ascontiguousarray(Wv[sl].T).astype(NPBF16).reshape(-1),
            np.ascontiguousarray(Wo[:, sl].T).astype(NPBF16).reshape(-1),
        ])
        assert blob.shape == (BLOB_SZ,)
        m = {"blob": blob}
        if use_bias:
            m["bq"] = np.ascontiguousarray(bq[sl]).reshape(HEADS_PC, 128)
            m["bk"] = np.ascontiguousarray(bk[sl]).reshape(HEADS_PC, 128)
            m["bv"] = np.ascontiguousarray(bv[sl]).reshape(1, HPC)
        if use_mask:
            mt = mask[:, 0].transpose(0, 2, 1) / SCALE  # [B, tk, tq]
            m["maskT"] = np.ascontiguousarray(mt).reshape(B, KT, 128, T)
        in_maps.append(m)
    return nc, in_maps


def postprocess(results, inputs):
    bo = np.asarray(inputs["bo"], dtype=np.float32)
    acc = results[0]["out"].astype(np.float32)
    for c in range(1, N_CORES):
        acc = acc + results[c]["out"].astype(np.float32)
    return (acc + bo).reshape(B, T, H)


def kernel(**inputs):
    import time as _time

    nc, in_maps = prepare(inputs)
    last_err = None
    for attempt in range(3):
        try:
            res = run_bass_kernel_spmd(nc, in_maps, list(range(N_CORES)))
            result = postprocess(res.results, inputs)
            if np.isfinite(result).all():
                return result
            # transient device fault can yield garbage without raising
            last_err = ValueError("non-finite kernel output")
        except Exception as e:
            last_err = e
        _time.sleep(2.0)
        try:  # best-effort device recovery before retrying
            import jax
            jax.extend.backend.clear_backends()
        except Exception:
            pass
    raise last_err
